# revision 43
# baseline (speedup 1.0000x reference)
"""Trainium2 Bass kernel for nn_DeepTimeGraphNet (per-row conv/pool pyramid + classifier).

Contract: kernel(**inputs) takes the FULL unsharded inputs (keys as in
setup_inputs()) and returns the FULL (64, 3) softmax output.

Two weight-specialized programs (chosen at build time, both SPMD over the
8 cores):

1. Constant-fold fast path. A sound interval analysis over x in
   (-inf, inf) propagates bounds through the conv/pool/relu pyramid
   (including conv zero-padding). For the reference weights, c4_w is
   all-negative, so y4 <= b4, r5 in [0, b4], and y6's upper bound is
   negative: r7 === 0 and the per-node feature is EXACTLY b8 for every
   possible input. When that proof succeeds the conv pyramid is dead code
   for ALL inputs (not just the given one) and the device only computes
   the classifier + softmax (~15us, dominated by fixed NEFF overhead).
2. Streaming fallback (proof fails): full fp16 PE-conv pipeline below.

Sharding: pure data parallel over batch. Core i handles batch rows
[8i, 8i+8) = 8192 (batch, node) rows of length 1200, processed as 8
supertiles of 1024 rows = 128 SBUF partitions x 8 row groups.

Streaming design ("PE conv + fp16 stream", ~85us vs 161us baseline):

- x is cast to fp16 ON HOST and fed to the device as fp16, halving HBM
  traffic (the memory-regime bottleneck): 19.66 MB/core -> ~48us DMA
  floor across 16 queues.
- Host pre-permutes each 1200-sample row into 6 contiguous phase
  planes (t = 6g+2j+e -> plane (j,e), g): conv0's two taps and the
  3-way maxpool phases become CONTIGUOUS views for the PE and DVE.
- conv0 (k2 s2) runs on the otherwise-idle PE as two accumulating
  matmuls per (phase, row-pair) with scaled-identity stationaries
  (w0*I then w1*I), fp16 moving at ~2.4 cols/ns into fp32 PSUM.
  Row-pair-granular single-bank PSUM tiles keep bank turnaround fast.
- maxpool3: DVE has a single PSUM read port, so ScalarE copies phase 0
  to SBUF (fp16), then DVE TT max + STT max against the other two PSUM
  phases; relu+b0 in one contiguous ScalarE ACT per half -> r1.
- conv2 (k4 s2 p1) on PE: 4 accumulating matmuls (w2_k*I) with
  STRIDED moving views of r1 (no deinterleave pass needed).
  maxpool2+relu+b2 -> r3 via ScalarE PSUM-copy + DVE TT + ScalarE ACT.
- conv4/conv6/conv8 + pools batched on DVE/ScalarE over supertile
  pairs; batches scheduled as early as dependencies allow so the drain
  tail stays short.
- classifier + exact softmax as v3 (stt row-dots + PE matmul with
  per-batch partition-block masks).
"""
import os
import sys

for _p in ("/root/.axon_site/_ro/trn_rl_repo", "/opt/trn_rl_repo"):
    if os.path.isdir(_p) and _p not in sys.path:
        sys.path.insert(0, _p)

import numpy as np  # noqa: E402

import concourse.bacc as bacc  # noqa: E402
import concourse.tile as tile  # noqa: E402
from concourse import mybir  # noqa: E402
from concourse.bass_utils import run_bass_kernel_spmd  # noqa: E402

F32 = mybir.dt.float32
F16 = mybir.dt.float16
Alu = mybir.AluOpType
Act = mybir.ActivationFunctionType

BS, NN, T = 64, 1024, 1200
N_CORES = 8
S_PER_CORE = 8          # supertiles per core; each = 1024 rows
C = 8                   # row groups per supertile (128 partitions x 8 rows)

_CACHE = {}


def _build(w):
    """Build + compile the per-core SPMD program with weights baked in."""
    nc = bacc.Bacc("TRN2", target_bir_lowering=False, debug=False)
    x = nc.dram_tensor("x", [S_PER_CORE * C * 128, T], F16, kind="ExternalInput")
    wid = nc.dram_tensor("wid", [128, 6 * 128], F16, kind="ExternalInput")
    clswt = nc.dram_tensor("clswt", [128, 200], F32, kind="ExternalInput")
    out = nc.dram_tensor("out", [8, 3], F32, kind="ExternalOutput")

    w4, w6, w8 = w["w4"], w["w6"], w["w8"]
    stt = nc.vector.scalar_tensor_tensor

    # x rows are ordered (p s c): row = 64p + 8s + c. Within a row the 1200
    # samples are host-permuted to 6 planes: col = (2j+e)*200 + g where the
    # original t = 6g + 2j + e.
    xsrc = x[:].rearrange("(p s c) t -> s p (c t)", p=128, s=S_PER_CORE, c=C)

    with tile.TileContext(nc) as tc:
        with (
            tc.tile_pool(name="xpool", bufs=3) as xpool,
            tc.tile_pool(name="m1p", bufs=6) as m1p,
            tc.tile_pool(name="m2p", bufs=3) as m2p,
            tc.tile_pool(name="r1p", bufs=3) as r1p,
            tc.tile_pool(name="m4p", bufs=4) as m4p,
            tc.tile_pool(name="const", bufs=1) as const,
            tc.tile_pool(name="zps", bufs=6, space="PSUM") as zps,
            tc.tile_pool(name="yps", bufs=2, space="PSUM") as yps,
        ):
            idw = const.tile([128, 6 * 128], F16)
            clsw = const.tile([128, 200], F32)
            featmat = const.tile([128, 64], F32)
            biases = const.tile([128, 2], F32)

            # persistent staging for the batched small stages
            r3all = const.tile([128, S_PER_CORE * C * 50], F16)
            y4all = const.tile([128, S_PER_CORE * C * 25], F32)
            r5all = const.tile([128, S_PER_CORE * C * 12], F16)
            y6all = const.tile([128, S_PER_CORE * C * 6], F32)
            r7all = const.tile([128, S_PER_CORE * C * 3], F16)
            fball = const.tile([128, S_PER_CORE * C], F32)
            r3v = r3all[:].rearrange("p (s c t) -> p s c t", s=S_PER_CORE, c=C)
            y4v = y4all[:].rearrange("p (s c t) -> p s c t", s=S_PER_CORE, c=C)
            r5v = r5all[:].rearrange("p (s c t) -> p s c t", s=S_PER_CORE, c=C)
            y6v = y6all[:].rearrange("p (s c t) -> p s c t", s=S_PER_CORE, c=C)
            r7v = r7all[:].rearrange("p (s c t) -> p s c t", s=S_PER_CORE, c=C)
            fbv = fball[:].rearrange("p (s c) -> p s c", s=S_PER_CORE)
            fmv = featmat[:].rearrange("p (s c) -> p s c", s=S_PER_CORE)

            def W(k):
                return idw[:, 128 * k:128 * (k + 1)]

            st = {}

            def conv0_rp(s, h, a, m2):
                """PE: y0 phase planes for row pair (4h+2a, 4h+2a+1); ScalarE
                copies phase 0 out of PSUM (DVE has one PSUM read port);
                DVE pools the 3 phases + relu later via ScalarE."""
                xt6 = st[s]["x"][:].rearrange("p (c pl g) -> p c pl g",
                                              c=C, pl=6)
                r0 = 4 * h + 2 * a
                zt = [zps.tile([128, 512], F32, name="zt") for j in range(3)]
                for j in range(3):
                    nc.tensor.matmul(zt[j][:, 0:400], W(0),
                                     xt6[:, r0:r0 + 2, 2 * j, :],
                                     start=True, stop=False)
                for j in range(3):
                    nc.tensor.matmul(zt[j][:, 0:400], W(1),
                                     xt6[:, r0:r0 + 2, 2 * j + 1, :],
                                     start=False, stop=True)
                    if j == 0:
                        zc = m1p.tile([128, 400], F16, name="zc")
                        nc.scalar.activation(zc[:], zt[0][:, 0:400], Act.Copy)
                    elif j == 1:
                        m1 = m1p.tile([128, 400], F16)
                        nc.vector.tensor_tensor(m1[:], zc[:], zt[1][:, 0:400],
                                                Alu.max)
                    else:
                        stt(m2[:, 400 * (2 * h + a):400 * (2 * h + a) + 400],
                            m1[:], 0.0, zt[2][:, 0:400], Alu.add, Alu.max)

            def relu_half(s, h, m2, r1):
                """ScalarE: r1 = relu(m2 + b0), contiguous."""
                cs = slice(800 * h, 800 * h + 800)
                nc.scalar.activation(r1[:, cs], m2[:, cs], Act.Relu,
                                     bias=biases[:, 0:1])

            def conv2_s4_half(s, h, r1):
                """PE: conv2 (4 taps, strided moving views of r1) -> y2 PSUM;
                ScalarE+DVE: maxpool2 + relu + b2 -> r3all."""
                r1v = r1[:].rearrange("p (c v) -> p c v", c=C)
                cs = slice(4 * h, 4 * h + 4)
                y2 = yps.tile([128, 512], F32)
                y2v = y2[:, 0:400].rearrange("p (c v) -> p c v", c=4)
                # y2[v] = w2_1*r1[2v] + w2_2*r1[2v+1] + w2_0*r1[2v-1]
                #         + w2_3*r1[2v+2]
                nc.tensor.matmul(y2[:, 0:400], W(3), r1v[:, cs, 0:200:2],
                                 start=True, stop=False)
                nc.tensor.matmul(y2[:, 0:400], W(4), r1v[:, cs, 1:200:2],
                                 start=False, stop=False)
                nc.tensor.matmul(y2v[:, :, 1:100], W(2), r1v[:, cs, 1:198:2],
                                 start=False, stop=False)
                nc.tensor.matmul(y2v[:, :, 0:99], W(5), r1v[:, cs, 2:199:2],
                                 start=False, stop=True)
                ce = m4p.tile([128, 200], F16, name="ce")
                nc.scalar.activation(ce[:].rearrange("p (c t) -> p c t", c=4),
                                     y2v[:, :, 0:100:2], Act.Copy)
                m4 = m4p.tile([128, 200], F16)
                nc.vector.tensor_tensor(
                    m4[:].rearrange("p (c t) -> p c t", c=4),
                    ce[:].rearrange("p (c t) -> p c t", c=4),
                    y2v[:, :, 1:100:2], Alu.max)
                nc.scalar.activation(r3v[:, s, cs, :],
                                     m4[:].rearrange("p (c t) -> p c t", c=4),
                                     Act.Relu, bias=biases[:, 1:2])

            def tail_batch(lo, hi):
                """conv4..conv8 + pools, batched over supertiles [lo, hi)."""
                sl = slice(lo, hi)
                R3 = r3v[:, sl]
                Y4 = y4v[:, sl]
                nc.scalar.activation(Y4, R3[:, :, :, 0:50:2], Act.Copy,
                                     bias=w["b4"], scale=w4[1])
                stt(Y4, R3[:, :, :, 1:50:2], w4[2], Y4, Alu.mult, Alu.add)
                stt(Y4[:, :, :, 1:25], R3[:, :, :, 1:48:2], w4[0],
                    Y4[:, :, :, 1:25], Alu.mult, Alu.add)
                stt(Y4[:, :, :, 0:24], R3[:, :, :, 2:49:2], w4[3],
                    Y4[:, :, :, 0:24], Alu.mult, Alu.add)
                R5 = r5v[:, sl]
                stt(R5, Y4[:, :, :, 0:24:2], 0.0, Y4[:, :, :, 1:25:2],
                    Alu.max, Alu.max)
                Y6 = y6v[:, sl]
                nc.scalar.activation(Y6, R5[:, :, :, 0:12:2], Act.Copy,
                                     bias=w["b6"], scale=w6[1])
                stt(Y6, R5[:, :, :, 1:12:2], w6[2], Y6, Alu.mult, Alu.add)
                stt(Y6[:, :, :, 1:6], R5[:, :, :, 1:10:2], w6[0],
                    Y6[:, :, :, 1:6], Alu.mult, Alu.add)
                stt(Y6[:, :, :, 0:5], R5[:, :, :, 2:11:2], w6[3],
                    Y6[:, :, :, 0:5], Alu.mult, Alu.add)
                R7 = r7v[:, sl]
                stt(R7, Y6[:, :, :, 0:6:2], 0.0, Y6[:, :, :, 1:6:2],
                    Alu.max, Alu.max)
                FB = fbv[:, sl]
                nc.scalar.activation(FB, R7[:, :, :, 0], Act.Copy,
                                     bias=w["b8"], scale=w8[0])
                stt(FB, R7[:, :, :, 1], w8[1], FB, Alu.mult, Alu.add)
                stt(fmv[:, sl], R7[:, :, :, 2], w8[2], FB, Alu.mult, Alu.add)

            def dma_supertile(s, quarters=(1, 1)):
                """DMA a supertile; `quarters` lists the split factor per
                half (2 entries -> finer chunks for fill/drain edges)."""
                xt = xpool.tile([128, C * T], F16)
                st[s] = {"x": xt}
                for h, q in enumerate(quarters):
                    for k in range(q):
                        w0 = h * 4800 + k * (4800 // q)
                        nc.sync.dma_start(xt[:, w0:w0 + 4800 // q],
                                          xsrc[s][:, w0:w0 + 4800 // q])

            # stationaries first (tiny, the PE's first dependency), then the
            # x stream with a quarter-split leading chunk.
            nc.sync.dma_start(idw[:], wid[:])
            dma_supertile(0, quarters=(2, 1))
            nc.vector.memset(biases[:, 0:1], w["b0"])
            nc.vector.memset(biases[:, 1:2], w["b2"])

            prevhalf = None
            for s in range(S_PER_CORE):
                if s + 1 < S_PER_CORE:
                    dma_supertile(s + 1, quarters=(2, 2))
                if s == 1:
                    nc.sync.dma_start(clsw[:], clswt[:])
                m2 = m2p.tile([128, C * 200], F16)
                r1 = r1p.tile([128, C * 200], F16)
                for h in range(2):
                    conv0_rp(s, h, 0, m2)
                    conv0_rp(s, h, 1, m2)
                    # previous half's relu+conv2 queue BEHIND this half's zc
                    # copies so PSUM banks recycle as early as possible
                    if prevhalf is not None:
                        ps, ph, pm2, pr1 = prevhalf
                        relu_half(ps, ph, pm2, pr1)
                        conv2_s4_half(ps, ph, pr1)
                    prevhalf = (s, h, m2, r1)
                if s >= 2:
                    st.pop(s - 2, None)
                if s == 3:
                    tail_batch(0, 2)
                elif s == 5:
                    tail_batch(2, 4)
                elif s == 6:
                    tail_batch(4, 6)
                elif s == 7:
                    tail_batch(6, 7)

            ps, ph, pm2, pr1 = prevhalf
            relu_half(ps, ph, pm2, pr1)
            conv2_s4_half(ps, ph, pr1)
            tail_batch(7, 8)

            # classifier: batch b = p//16; partial_j[p] = <feat[p,:], Wj[p,:]>
            # then PE matmul with the 0/1 block mask sums each 16-partition
            # block into logits[b, j].
            dum = const.tile([128, 64], F32)
            partial = const.tile([128, 3], F32)
            for j in range(3):
                stt(dum[:], featmat[:, 0:64], 1.0, clsw[:, j * 64:(j + 1) * 64],
                    Alu.mult, Alu.mult, accum_out=partial[:, j:j + 1])
            lg = yps.tile([8, 3], F32, name="lg", tag="y2")
            nc.tensor.matmul(lg[:], clsw[:, 192:200], partial[:],
                             start=True, stop=True)
            if any(v != 0.0 for v in w["cls_b"]):
                lgs = const.tile([8, 3], F32)
                nc.vector.tensor_copy(lgs[:], lg[:])
                for cls in range(3):
                    if w["cls_b"][cls] != 0.0:
                        nc.vector.tensor_scalar_add(lgs[:, cls:cls + 1],
                                                    lgs[:, cls:cls + 1],
                                                    w["cls_b"][cls])
                lsrc = lgs[:]
            else:
                lsrc = lg[:]   # zero bias: reduce + Exp read PSUM directly
            # softmax (max-subtracted, like jax.nn.softmax)
            nmx = const.tile([8, 1], F32)
            nc.vector.tensor_reduce(nmx[:], lsrc, mybir.AxisListType.X, Alu.max,
                                    negate=True)
            ex = const.tile([8, 3], F32)
            smv = const.tile([8, 1], F32)
            nc.scalar.activation(ex[:], lsrc, Act.Exp, bias=nmx[:], scale=1.0,
                                 accum_out=smv[:])
            ri = const.tile([8, 1], F32)
            nc.vector.reciprocal(ri[:], smv[:])
            pr = const.tile([8, 3], F32)
            nc.vector.tensor_scalar(pr[:], ex[:], ri[:], None, Alu.mult)
            nc.sync.dma_start(out[:], pr[:])

    nc.compile()
    return nc


def _interval_const_feat(w):
    """Sound interval analysis of the conv pyramid over x in (-inf, inf).

    Propagates [lo, hi] bounds through conv0..conv8 + pools + relus exactly
    as the reference computes them. If the interval collapses to a point
    (e.g. an all-nonpositive conv makes a later relu identically zero for
    EVERY real input), the per-node feature is a weight-dependent constant
    and the conv pyramid is dead code for all inputs. Returns that constant,
    or None when the bounds stay input-dependent.
    """
    inf = float("inf")

    def conv(iv, taps, b, padded):
        if padded:
            # boundary taps read zero-padding: the input hull must include 0
            iv = (min(iv[0], 0.0), max(iv[1], 0.0))
        lo = sum(min(t * iv[0], t * iv[1]) for t in taps) + b
        hi = sum(max(t * iv[0], t * iv[1]) for t in taps) + b
        return (lo, hi)

    relu = lambda iv: (max(0.0, iv[0]), max(0.0, iv[1]))
    # y0/pool3/relu: unbounded input -> r1 in [0, inf)
    r1 = (0.0, inf)
    r3 = relu(conv(r1, w["w2"], w["b2"], True))
    r5 = relu(conv(r3, w["w4"], w["b4"], True))
    r7 = relu(conv(r5, w["w6"], w["b6"], True))
    feat = conv(r7, w["w8"], w["b8"], False)
    if np.isfinite(feat[0]) and feat[0] == feat[1]:
        return float(feat[0])
    return None


def _build_const(w):
    """Tiny program for the constant-feature case: the device computes the
    classifier + softmax from the (proven constant) per-node feature.

    Input wsm [128, 24]: wsm[p, 3c+j] = cls_w[j, 8p+c] (node n = 8p+c), so
    partial[p, j] = cval*sum_c wsm[p, c, j] (+ cls_b[j]/1024 per element)
    via DVE tensor_scalar+accum, and a ones-stationary PE matmul sums the
    partitions into logits (identical for every batch).

    When the weight-only logit bound |lg| < 60 holds (checked at build
    time), softmax skips the max-subtract: exp can't overflow, and the
    shift-invariant result is unchanged. The final division runs as a
    ScalarE ACT (scale = reciprocal AP) so ScalarE itself issues the out
    DMA with no cross-engine hop.
    """
    nc = bacc.Bacc("TRN2", target_bir_lowering=False, debug=False)
    narrow = w.get("wst_narrow", False)
    if narrow:
        # 16-partition layout: one contiguous 780B descriptor per partition
        # (16 total vs 128) so the input DMA clears the DGE pipeline sooner.
        # wst[p, 65j+c] = cls_w[j, 64p+c] for c<64; col 65j+64 carries
        # cls_b[j]/(16*cval) so the cval-scaled matmul colsum adds cls_b once.
        wsmt = nc.dram_tensor("wsmt", [16, 195], F32, kind="ExternalInput")
    else:
        wsmt = nc.dram_tensor("wsmt", [128, 24], F32, kind="ExternalInput")
    out = nc.dram_tensor("out", [8, 3], F32, kind="ExternalOutput")
    cval = w["const_feat"]
    with tile.TileContext(nc) as tc:
        with (
            tc.tile_pool(name="const", bufs=1) as const,
            tc.tile_pool(name="psum", bufs=1, space="PSUM") as psum,
        ):
            lg = psum.tile([8, 3], F32)
            if narrow:
                wst = const.tile([16, 195], F32)
                # ScalarE-issued measured ~0.5us faster than SP-issued in
                # clean 4-run clusters (14.44 vs 14.97 median), despite the
                # act-func-set preamble in ScalarE's queue.
                nc.scalar.dma_start(wst[:], wsmt[:])
                cones = const.tile([16, 8], F32)
                nc.vector.memset(cones[:], cval)
                partial = const.tile([16, 3], F32)
                nc.vector.tensor_reduce(
                    partial[:].rearrange("p (j o) -> p j o", o=1),
                    wst[:].rearrange("p (j c) -> p j c", j=3),
                    mybir.AxisListType.X, Alu.add)
                nc.tensor.matmul(lg[:], cones[:], partial[:],
                                 start=True, stop=True)
            else:
                wsm = const.tile([128, 24], F32)
                nc.scalar.dma_start(wsm[:], wsmt[:])
                ones8 = const.tile([128, 8], F32)
                nc.vector.memset(ones8[:], 1.0)
                wsv = wsm[:].rearrange("p (c j) -> p c j", c=8)
                dum = const.tile([128, 8], F32)
                partial = const.tile([128, 3], F32)
                for j in range(3):
                    nc.vector.tensor_scalar(
                        dum[:], wsv[:, :, j], cval, w["cls_b"][j] / 1024.0,
                        Alu.mult, Alu.add, accum_out=partial[:, j:j + 1])
                nc.tensor.matmul(lg[:], ones8[:], partial[:],
                                 start=True, stop=True)
            ex = const.tile([8, 3], F32)
            smv = const.tile([8, 1], F32)
            if w.get("lg_bounded", False):
                nc.scalar.activation(ex[:], lg[:], Act.Exp, bias=0.0,
                                     scale=1.0, accum_out=smv[:])
            else:
                nmx = const.tile([8, 1], F32)
                nc.vector.tensor_reduce(nmx[:], lg[:], mybir.AxisListType.X,
                                        Alu.max, negate=True)
                nc.scalar.activation(ex[:], lg[:], Act.Exp, bias=nmx[:],
                                     scale=1.0, accum_out=smv[:])
            ri = const.tile([8, 1], F32)
            nc.vector.reciprocal(ri[:], smv[:])
            pr = const.tile([8, 3], F32)
            nc.scalar.activation(pr[:], ex[:], Act.Copy, scale=ri[:])
            nc.scalar.dma_start(out[:], pr[:])
    nc.compile()
    return nc


def _extract_weights(inputs):
    f = lambda a: [float(v) for v in np.asarray(a).reshape(-1)]
    return dict(
        w00=f(inputs["c0_w"])[0], w01=f(inputs["c0_w"])[1], b0=f(inputs["c0_b"])[0],
        w2=f(inputs["c2_w"]), b2=f(inputs["c2_b"])[0],
        w4=f(inputs["c4_w"]), b4=f(inputs["c4_b"])[0],
        w6=f(inputs["c6_w"]), b6=f(inputs["c6_b"])[0],
        w8=f(inputs["c8_w"]), b8=f(inputs["c8_b"])[0],
        cls_b=f(inputs["cls_b"]),
    )


def _make_clsT(cls_w):
    """Classifier weights in the device layout. Consecutive-row mapping:
    feat[p, s, c] is dram row 64p + 8s + c -> node 64*(p%16) + 8s + c,
    batch block b = p//16; cols 192:200 hold the 0/1 block mask."""
    clsT = np.zeros((128, 200), np.float32)
    pidx = np.arange(128)
    node = (64 * (pidx % 16))[:, None] + np.arange(64)[None, :]   # [p, s*8+c]
    for j in range(3):
        clsT[:, j * 64:(j + 1) * 64] = cls_w[j][node]
    clsT[pidx, 192 + pidx // 16] = 1.0
    return clsT


def _prep_x(x):
    """fp32 (BS*NN, T) -> fp16 with each row permuted into 6 phase planes:
    col = (2j+e)*200 + g for original t = 6g + 2j + e."""
    x16 = np.asarray(x, dtype=np.float32).reshape(BS * NN, 200, 3, 2)
    x16 = x16.transpose(0, 2, 3, 1).astype(np.float16)
    return np.ascontiguousarray(x16).reshape(BS * NN, T)


def _run(inputs, trace=False, trace_kwargs=None, allow_const=True):
    w = _extract_weights(inputs)
    const_feat = _interval_const_feat(w) if allow_const else None
    w["const_feat"] = const_feat
    cls_w_ = np.asarray(inputs["cls_w"], dtype=np.float32)
    if const_feat is not None:
        # weight-only logit bound: safe to skip softmax's max-subtract?
        lg_host = const_feat * cls_w_.astype(np.float64).sum(axis=1) \
            + np.asarray(w["cls_b"], np.float64)
        w["lg_bounded"] = bool(np.all(np.isfinite(lg_host))
                               and np.abs(lg_host).max() < 60.0)
        w["wst_narrow"] = bool(abs(const_feat) > 1e-20
                               and np.isfinite(3.0 / const_feat))
    key = tuple(np.asarray(
        [w["w00"], w["w01"], w["b0"]] + w["w2"] + [w["b2"]] + w["w4"] + [w["b4"]]
        + w["w6"] + [w["b6"]] + w["w8"] + [w["b8"]] + w["cls_b"]
        + [0.0 if const_feat is None else
           (2.0 if w.get("lg_bounded") else 1.0)
           + (4.0 if w.get("wst_narrow") else 0.0)],
        np.float64
    ).tobytes())
    if key not in _CACHE:
        _CACHE[key] = _build_const(w) if const_feat is not None else _build(w)
    nc = _CACHE[key]

    if const_feat is not None:
        # conv pyramid proven constant for ALL inputs with these weights:
        # device computes classifier + softmax only.
        if w["wst_narrow"]:
            wsm = np.zeros((16, 195), np.float32)
            wj = cls_w_.reshape(3, 16, 64).transpose(1, 0, 2)   # [p, j, c]
            for j in range(3):
                wsm[:, 65 * j:65 * j + 64] = wj[:, j, :]
                wsm[:, 65 * j + 64] = np.float32(
                    w["cls_b"][j] / (16.0 * const_feat))
        else:
            wsm = np.ascontiguousarray(
                cls_w_.reshape(3, 128, 8).transpose(1, 2, 0).reshape(128, 24))
        in_maps = [{"wsmt": wsm} for _ in range(N_CORES)]
        res = run_bass_kernel_spmd(nc, in_maps, list(range(N_CORES)),
                                   trace=trace, **(trace_kwargs or {}))
        out = np.concatenate(
            [np.asarray(res.results[i]["out"]) for i in range(N_CORES)],
            axis=0).astype(np.float32)
        return out, res

    xp = _prep_x(np.asarray(inputs["x"], dtype=np.float32).reshape(BS * NN, T))

    # scaled-identity stationaries: w0*I, w1*I, w2_k*I
    wid = np.zeros((128, 6 * 128), np.float16)
    ar = np.arange(128)
    for k, val in enumerate([w["w00"], w["w01"]] + list(w["w2"])):
        wid[ar, 128 * k + ar] = np.float16(val)

    clsT = _make_clsT(cls_w_)

    rows_per_core = BS * NN // N_CORES
    in_maps = [
        {"x": np.ascontiguousarray(xp[i * rows_per_core:(i + 1) * rows_per_core]),
         "wid": wid, "clswt": clsT}
        for i in range(N_CORES)
    ]
    res = run_bass_kernel_spmd(nc, in_maps, list(range(N_CORES)), trace=trace,
                               **(trace_kwargs or {}))
    out = np.concatenate([np.asarray(res.results[i]["out"]) for i in range(N_CORES)],
                         axis=0).astype(np.float32)
    return out, res


def kernel(**inputs):
    out, _ = _run(inputs, trace=False)
    return out


# revision 46
# speedup vs baseline: 1.0369x; 1.0369x over previous
"""Trainium2 Bass kernel for nn_DeepTimeGraphNet (per-row conv/pool pyramid + classifier).

Contract: kernel(**inputs) takes the FULL unsharded inputs (keys as in
setup_inputs()) and returns the FULL (64, 3) softmax output.

Two weight-specialized programs (chosen at build time, both SPMD over the
8 cores):

1. Constant-fold fast path. A sound interval analysis over x in
   (-inf, inf) propagates bounds through the conv/pool/relu pyramid
   (including conv zero-padding). For the reference weights, c4_w is
   all-negative, so y4 <= b4, r5 in [0, b4], and y6's upper bound is
   negative: r7 === 0 and the per-node feature is EXACTLY b8 for every
   possible input. When that proof succeeds the conv pyramid is dead code
   for ALL inputs (not just the given one) and the device only computes
   the classifier + softmax (~14.4us, dominated by fixed NEFF overhead:
   ~5.8us runtime preamble + 1.3us act-func-set + DGE latencies + ~2.5us
   teardown drains).
2. Streaming fallback (proof fails): full fp16 PE-conv pipeline below.

Sharding: pure data parallel over batch. Core i handles batch rows
[8i, 8i+8) = 8192 (batch, node) rows of length 1200, processed as 8
supertiles of 1024 rows = 128 SBUF partitions x 8 row groups.

Streaming design ("PE conv + fp16 stream", ~85us vs 161us baseline):

- x is cast to fp16 ON HOST and fed to the device as fp16, halving HBM
  traffic (the memory-regime bottleneck): 19.66 MB/core -> ~48us DMA
  floor across 16 queues.
- Host pre-permutes each 1200-sample row into 6 contiguous phase
  planes (t = 6g+2j+e -> plane (j,e), g): conv0's two taps and the
  3-way maxpool phases become CONTIGUOUS views for the PE and DVE.
- conv0 (k2 s2) runs on the otherwise-idle PE as two accumulating
  matmuls per (phase, row-pair) with scaled-identity stationaries
  (w0*I then w1*I), fp16 moving at ~2.4 cols/ns into fp32 PSUM.
  Row-pair-granular single-bank PSUM tiles keep bank turnaround fast.
- maxpool3: DVE has a single PSUM read port, so ScalarE copies phase 0
  to SBUF (fp16), then DVE TT max + STT max against the other two PSUM
  phases; relu+b0 in one contiguous ScalarE ACT per half -> r1.
- conv2 (k4 s2 p1) on PE: 4 accumulating matmuls (w2_k*I) with
  STRIDED moving views of r1 (no deinterleave pass needed).
  maxpool2+relu+b2 -> r3 via ScalarE PSUM-copy + DVE TT + ScalarE ACT.
- conv4/conv6/conv8 + pools batched on DVE/ScalarE over supertile
  pairs; batches scheduled as early as dependencies allow so the drain
  tail stays short.
- classifier + exact softmax as v3 (stt row-dots + PE matmul with
  per-batch partition-block masks).
"""
import os
import sys

for _p in ("/root/.axon_site/_ro/trn_rl_repo", "/opt/trn_rl_repo"):
    if os.path.isdir(_p) and _p not in sys.path:
        sys.path.insert(0, _p)

import numpy as np  # noqa: E402

import concourse.bacc as bacc  # noqa: E402
import concourse.tile as tile  # noqa: E402
from concourse import mybir  # noqa: E402
from concourse.bass_utils import run_bass_kernel_spmd  # noqa: E402

F32 = mybir.dt.float32
F16 = mybir.dt.float16
Alu = mybir.AluOpType
Act = mybir.ActivationFunctionType

BS, NN, T = 64, 1024, 1200
N_CORES = 8
S_PER_CORE = 8          # supertiles per core; each = 1024 rows
C = 8                   # row groups per supertile (128 partitions x 8 rows)

_CACHE = {}


def _build(w):
    """Build + compile the per-core SPMD program with weights baked in."""
    nc = bacc.Bacc("TRN2", target_bir_lowering=False, debug=False)
    x = nc.dram_tensor("x", [S_PER_CORE * C * 128, T], F16, kind="ExternalInput")
    wid = nc.dram_tensor("wid", [128, 6 * 128], F16, kind="ExternalInput")
    clswt = nc.dram_tensor("clswt", [128, 200], F32, kind="ExternalInput")
    out = nc.dram_tensor("out", [8, 3], F32, kind="ExternalOutput")

    w4, w6, w8 = w["w4"], w["w6"], w["w8"]
    stt = nc.vector.scalar_tensor_tensor

    # x rows are ordered (p s c): row = 64p + 8s + c. Within a row the 1200
    # samples are host-permuted to 6 planes: col = (2j+e)*200 + g where the
    # original t = 6g + 2j + e.
    xsrc = x[:].rearrange("(p s c) t -> s p (c t)", p=128, s=S_PER_CORE, c=C)

    with tile.TileContext(nc) as tc:
        with (
            tc.tile_pool(name="xpool", bufs=3) as xpool,
            tc.tile_pool(name="m1p", bufs=6) as m1p,
            tc.tile_pool(name="m2p", bufs=3) as m2p,
            tc.tile_pool(name="r1p", bufs=3) as r1p,
            tc.tile_pool(name="m4p", bufs=4) as m4p,
            tc.tile_pool(name="const", bufs=1) as const,
            tc.tile_pool(name="zps", bufs=6, space="PSUM") as zps,
            tc.tile_pool(name="yps", bufs=2, space="PSUM") as yps,
        ):
            idw = const.tile([128, 6 * 128], F16)
            clsw = const.tile([128, 200], F32)
            featmat = const.tile([128, 64], F32)
            biases = const.tile([128, 2], F32)

            # persistent staging for the batched small stages
            r3all = const.tile([128, S_PER_CORE * C * 50], F16)
            y4all = const.tile([128, S_PER_CORE * C * 25], F32)
            r5all = const.tile([128, S_PER_CORE * C * 12], F16)
            y6all = const.tile([128, S_PER_CORE * C * 6], F32)
            r7all = const.tile([128, S_PER_CORE * C * 3], F16)
            fball = const.tile([128, S_PER_CORE * C], F32)
            r3v = r3all[:].rearrange("p (s c t) -> p s c t", s=S_PER_CORE, c=C)
            y4v = y4all[:].rearrange("p (s c t) -> p s c t", s=S_PER_CORE, c=C)
            r5v = r5all[:].rearrange("p (s c t) -> p s c t", s=S_PER_CORE, c=C)
            y6v = y6all[:].rearrange("p (s c t) -> p s c t", s=S_PER_CORE, c=C)
            r7v = r7all[:].rearrange("p (s c t) -> p s c t", s=S_PER_CORE, c=C)
            fbv = fball[:].rearrange("p (s c) -> p s c", s=S_PER_CORE)
            fmv = featmat[:].rearrange("p (s c) -> p s c", s=S_PER_CORE)

            def W(k):
                return idw[:, 128 * k:128 * (k + 1)]

            st = {}

            def conv0_rp(s, h, a, m2):
                """PE: y0 phase planes for row pair (4h+2a, 4h+2a+1); ScalarE
                copies phase 0 out of PSUM (DVE has one PSUM read port);
                DVE pools the 3 phases + relu later via ScalarE."""
                xt6 = st[s]["x"][:].rearrange("p (c pl g) -> p c pl g",
                                              c=C, pl=6)
                r0 = 4 * h + 2 * a
                zt = [zps.tile([128, 512], F32, name="zt") for j in range(3)]
                for j in range(3):
                    nc.tensor.matmul(zt[j][:, 0:400], W(0),
                                     xt6[:, r0:r0 + 2, 2 * j, :],
                                     start=True, stop=False)
                for j in range(3):
                    nc.tensor.matmul(zt[j][:, 0:400], W(1),
                                     xt6[:, r0:r0 + 2, 2 * j + 1, :],
                                     start=False, stop=True)
                    if j == 0:
                        zc = m1p.tile([128, 400], F16, name="zc")
                        nc.scalar.activation(zc[:], zt[0][:, 0:400], Act.Copy)
                    elif j == 1:
                        m1 = m1p.tile([128, 400], F16)
                        nc.vector.tensor_tensor(m1[:], zc[:], zt[1][:, 0:400],
                                                Alu.max)
                    else:
                        stt(m2[:, 400 * (2 * h + a):400 * (2 * h + a) + 400],
                            m1[:], 0.0, zt[2][:, 0:400], Alu.add, Alu.max)

            def relu_half(s, h, m2, r1):
                """ScalarE: r1 = relu(m2 + b0), contiguous."""
                cs = slice(800 * h, 800 * h + 800)
                nc.scalar.activation(r1[:, cs], m2[:, cs], Act.Relu,
                                     bias=biases[:, 0:1])

            def conv2_s4_half(s, h, r1):
                """PE: conv2 (4 taps, strided moving views of r1) -> y2 PSUM;
                ScalarE+DVE: maxpool2 + relu + b2 -> r3all."""
                r1v = r1[:].rearrange("p (c v) -> p c v", c=C)
                cs = slice(4 * h, 4 * h + 4)
                y2 = yps.tile([128, 512], F32)
                y2v = y2[:, 0:400].rearrange("p (c v) -> p c v", c=4)
                # y2[v] = w2_1*r1[2v] + w2_2*r1[2v+1] + w2_0*r1[2v-1]
                #         + w2_3*r1[2v+2]
                nc.tensor.matmul(y2[:, 0:400], W(3), r1v[:, cs, 0:200:2],
                                 start=True, stop=False)
                nc.tensor.matmul(y2[:, 0:400], W(4), r1v[:, cs, 1:200:2],
                                 start=False, stop=False)
                nc.tensor.matmul(y2v[:, :, 1:100], W(2), r1v[:, cs, 1:198:2],
                                 start=False, stop=False)
                nc.tensor.matmul(y2v[:, :, 0:99], W(5), r1v[:, cs, 2:199:2],
                                 start=False, stop=True)
                ce = m4p.tile([128, 200], F16, name="ce")
                nc.scalar.activation(ce[:].rearrange("p (c t) -> p c t", c=4),
                                     y2v[:, :, 0:100:2], Act.Copy)
                m4 = m4p.tile([128, 200], F16)
                nc.vector.tensor_tensor(
                    m4[:].rearrange("p (c t) -> p c t", c=4),
                    ce[:].rearrange("p (c t) -> p c t", c=4),
                    y2v[:, :, 1:100:2], Alu.max)
                nc.scalar.activation(r3v[:, s, cs, :],
                                     m4[:].rearrange("p (c t) -> p c t", c=4),
                                     Act.Relu, bias=biases[:, 1:2])

            def tail_batch(lo, hi):
                """conv4..conv8 + pools, batched over supertiles [lo, hi)."""
                sl = slice(lo, hi)
                R3 = r3v[:, sl]
                Y4 = y4v[:, sl]
                nc.scalar.activation(Y4, R3[:, :, :, 0:50:2], Act.Copy,
                                     bias=w["b4"], scale=w4[1])
                stt(Y4, R3[:, :, :, 1:50:2], w4[2], Y4, Alu.mult, Alu.add)
                stt(Y4[:, :, :, 1:25], R3[:, :, :, 1:48:2], w4[0],
                    Y4[:, :, :, 1:25], Alu.mult, Alu.add)
                stt(Y4[:, :, :, 0:24], R3[:, :, :, 2:49:2], w4[3],
                    Y4[:, :, :, 0:24], Alu.mult, Alu.add)
                R5 = r5v[:, sl]
                stt(R5, Y4[:, :, :, 0:24:2], 0.0, Y4[:, :, :, 1:25:2],
                    Alu.max, Alu.max)
                Y6 = y6v[:, sl]
                nc.scalar.activation(Y6, R5[:, :, :, 0:12:2], Act.Copy,
                                     bias=w["b6"], scale=w6[1])
                stt(Y6, R5[:, :, :, 1:12:2], w6[2], Y6, Alu.mult, Alu.add)
                stt(Y6[:, :, :, 1:6], R5[:, :, :, 1:10:2], w6[0],
                    Y6[:, :, :, 1:6], Alu.mult, Alu.add)
                stt(Y6[:, :, :, 0:5], R5[:, :, :, 2:11:2], w6[3],
                    Y6[:, :, :, 0:5], Alu.mult, Alu.add)
                R7 = r7v[:, sl]
                stt(R7, Y6[:, :, :, 0:6:2], 0.0, Y6[:, :, :, 1:6:2],
                    Alu.max, Alu.max)
                FB = fbv[:, sl]
                nc.scalar.activation(FB, R7[:, :, :, 0], Act.Copy,
                                     bias=w["b8"], scale=w8[0])
                stt(FB, R7[:, :, :, 1], w8[1], FB, Alu.mult, Alu.add)
                stt(fmv[:, sl], R7[:, :, :, 2], w8[2], FB, Alu.mult, Alu.add)

            def dma_supertile(s, quarters=(1, 1)):
                """DMA a supertile; `quarters` lists the split factor per
                half (2 entries -> finer chunks for fill/drain edges)."""
                xt = xpool.tile([128, C * T], F16)
                st[s] = {"x": xt}
                for h, q in enumerate(quarters):
                    for k in range(q):
                        w0 = h * 4800 + k * (4800 // q)
                        nc.sync.dma_start(xt[:, w0:w0 + 4800 // q],
                                          xsrc[s][:, w0:w0 + 4800 // q])

            # stationaries first (tiny, the PE's first dependency), then the
            # x stream with a quarter-split leading chunk.
            nc.sync.dma_start(idw[:], wid[:])
            dma_supertile(0, quarters=(2, 1))
            nc.vector.memset(biases[:, 0:1], w["b0"])
            nc.vector.memset(biases[:, 1:2], w["b2"])

            prevhalf = None
            for s in range(S_PER_CORE):
                if s + 1 < S_PER_CORE:
                    dma_supertile(s + 1, quarters=(2, 2))
                if s == 1:
                    nc.sync.dma_start(clsw[:], clswt[:])
                m2 = m2p.tile([128, C * 200], F16)
                r1 = r1p.tile([128, C * 200], F16)
                for h in range(2):
                    conv0_rp(s, h, 0, m2)
                    conv0_rp(s, h, 1, m2)
                    # previous half's relu+conv2 queue BEHIND this half's zc
                    # copies so PSUM banks recycle as early as possible
                    if prevhalf is not None:
                        ps, ph, pm2, pr1 = prevhalf
                        relu_half(ps, ph, pm2, pr1)
                        conv2_s4_half(ps, ph, pr1)
                    prevhalf = (s, h, m2, r1)
                if s >= 2:
                    st.pop(s - 2, None)
                if s == 3:
                    tail_batch(0, 2)
                elif s == 5:
                    tail_batch(2, 4)
                elif s == 6:
                    tail_batch(4, 6)
                elif s == 7:
                    tail_batch(6, 7)

            ps, ph, pm2, pr1 = prevhalf
            relu_half(ps, ph, pm2, pr1)
            conv2_s4_half(ps, ph, pr1)
            tail_batch(7, 8)

            # classifier: batch b = p//16; partial_j[p] = <feat[p,:], Wj[p,:]>
            # then PE matmul with the 0/1 block mask sums each 16-partition
            # block into logits[b, j].
            dum = const.tile([128, 64], F32)
            partial = const.tile([128, 3], F32)
            for j in range(3):
                stt(dum[:], featmat[:, 0:64], 1.0, clsw[:, j * 64:(j + 1) * 64],
                    Alu.mult, Alu.mult, accum_out=partial[:, j:j + 1])
            lg = yps.tile([8, 3], F32, name="lg", tag="y2")
            nc.tensor.matmul(lg[:], clsw[:, 192:200], partial[:],
                             start=True, stop=True)
            if any(v != 0.0 for v in w["cls_b"]):
                lgs = const.tile([8, 3], F32)
                nc.vector.tensor_copy(lgs[:], lg[:])
                for cls in range(3):
                    if w["cls_b"][cls] != 0.0:
                        nc.vector.tensor_scalar_add(lgs[:, cls:cls + 1],
                                                    lgs[:, cls:cls + 1],
                                                    w["cls_b"][cls])
                lsrc = lgs[:]
            else:
                lsrc = lg[:]   # zero bias: reduce + Exp read PSUM directly
            # softmax (max-subtracted, like jax.nn.softmax)
            nmx = const.tile([8, 1], F32)
            nc.vector.tensor_reduce(nmx[:], lsrc, mybir.AxisListType.X, Alu.max,
                                    negate=True)
            ex = const.tile([8, 3], F32)
            smv = const.tile([8, 1], F32)
            nc.scalar.activation(ex[:], lsrc, Act.Exp, bias=nmx[:], scale=1.0,
                                 accum_out=smv[:])
            ri = const.tile([8, 1], F32)
            nc.vector.reciprocal(ri[:], smv[:])
            pr = const.tile([8, 3], F32)
            nc.vector.tensor_scalar(pr[:], ex[:], ri[:], None, Alu.mult)
            nc.sync.dma_start(out[:], pr[:])

    nc.compile()
    return nc


def _interval_const_feat(w):
    """Sound interval analysis of the conv pyramid over x in (-inf, inf).

    Propagates [lo, hi] bounds through conv0..conv8 + pools + relus exactly
    as the reference computes them. If the interval collapses to a point
    (e.g. an all-nonpositive conv makes a later relu identically zero for
    EVERY real input), the per-node feature is a weight-dependent constant
    and the conv pyramid is dead code for all inputs. Returns that constant,
    or None when the bounds stay input-dependent.
    """
    inf = float("inf")

    def conv(iv, taps, b, padded):
        if padded:
            # boundary taps read zero-padding: the input hull must include 0
            iv = (min(iv[0], 0.0), max(iv[1], 0.0))
        lo = sum(min(t * iv[0], t * iv[1]) for t in taps) + b
        hi = sum(max(t * iv[0], t * iv[1]) for t in taps) + b
        return (lo, hi)

    relu = lambda iv: (max(0.0, iv[0]), max(0.0, iv[1]))
    # y0/pool3/relu: unbounded input -> r1 in [0, inf)
    r1 = (0.0, inf)
    r3 = relu(conv(r1, w["w2"], w["b2"], True))
    r5 = relu(conv(r3, w["w4"], w["b4"], True))
    r7 = relu(conv(r5, w["w6"], w["b6"], True))
    feat = conv(r7, w["w8"], w["b8"], False)
    if np.isfinite(feat[0]) and feat[0] == feat[1]:
        return float(feat[0])
    return None


def _build_const(w):
    """Tiny program for the constant-feature case: the device computes the
    classifier + softmax from the (proven constant) per-node feature.

    Input wsm [128, 24]: wsm[p, 3c+j] = cls_w[j, 8p+c] (node n = 8p+c), so
    partial[p, j] = cval*sum_c wsm[p, c, j] (+ cls_b[j]/1024 per element)
    via DVE tensor_scalar+accum, and a ones-stationary PE matmul sums the
    partitions into logits (identical for every batch).

    When the weight-only logit bound |lg| < 60 holds (checked at build
    time), softmax skips the max-subtract: exp can't overflow, and the
    shift-invariant result is unchanged. The final division runs as a
    ScalarE ACT (scale = reciprocal AP) so ScalarE itself issues the out
    DMA with no cross-engine hop.
    """
    nc = bacc.Bacc("TRN2", target_bir_lowering=False, debug=False)
    narrow = w.get("wst_narrow", False)
    if narrow:
        # 16-partition layout: one contiguous 780B descriptor per partition
        # (16 total vs 128) so the input DMA clears the DGE pipeline sooner.
        # wst[p, 65j+c] = cls_w[j, 64p+c] for c<64; col 65j+64 carries
        # cls_b[j]/(16*cval) so the cval-scaled matmul colsum adds cls_b once.
        wsmt = nc.dram_tensor("wsmt", [16, 195], F32, kind="ExternalInput")
    else:
        wsmt = nc.dram_tensor("wsmt", [128, 24], F32, kind="ExternalInput")
    out = nc.dram_tensor("out", [8, 3], F32, kind="ExternalOutput")
    cval = w["const_feat"]
    with tile.TileContext(nc) as tc:
        with (
            tc.tile_pool(name="const", bufs=1) as const,
            tc.tile_pool(name="psum", bufs=1, space="PSUM") as psum,
        ):
            lg = psum.tile([8, 3], F32)
            if narrow:
                wst = const.tile([16, 195], F32)
                # ScalarE-issued measured ~0.5us faster than SP-issued in
                # clean 4-run clusters (14.44 vs 14.97 median), despite the
                # act-func-set preamble in ScalarE's queue.
                nc.scalar.dma_start(wst[:], wsmt[:])
                cones = const.tile([16, 8], F32)
                nc.vector.memset(cones[:], cval)
                partial = const.tile([16, 3], F32)
                nc.vector.tensor_reduce(
                    partial[:].rearrange("p (j o) -> p j o", o=1),
                    wst[:].rearrange("p (j c) -> p j c", j=3),
                    mybir.AxisListType.X, Alu.add)
                nc.tensor.matmul(lg[:], cones[:], partial[:],
                                 start=True, stop=True)
            else:
                wsm = const.tile([128, 24], F32)
                nc.scalar.dma_start(wsm[:], wsmt[:])
                ones8 = const.tile([128, 8], F32)
                nc.vector.memset(ones8[:], 1.0)
                wsv = wsm[:].rearrange("p (c j) -> p c j", c=8)
                dum = const.tile([128, 8], F32)
                partial = const.tile([128, 3], F32)
                for j in range(3):
                    nc.vector.tensor_scalar(
                        dum[:], wsv[:, :, j], cval, w["cls_b"][j] / 1024.0,
                        Alu.mult, Alu.add, accum_out=partial[:, j:j + 1])
                nc.tensor.matmul(lg[:], ones8[:], partial[:],
                                 start=True, stop=True)
            ex = const.tile([8, 3], F32)
            smv = const.tile([8, 1], F32)
            if w.get("lg_bounded", False):
                nc.scalar.activation(ex[:], lg[:], Act.Exp, bias=0.0,
                                     scale=1.0, accum_out=smv[:])
            else:
                nmx = const.tile([8, 1], F32)
                nc.vector.tensor_reduce(nmx[:], lg[:], mybir.AxisListType.X,
                                        Alu.max, negate=True)
                nc.scalar.activation(ex[:], lg[:], Act.Exp, bias=nmx[:],
                                     scale=1.0, accum_out=smv[:])
            ri = const.tile([8, 1], F32)
            nc.vector.reciprocal(ri[:], smv[:])
            pr = const.tile([8, 3], F32)
            nc.scalar.activation(pr[:], ex[:], Act.Copy, scale=ri[:])
            nc.scalar.dma_start(out[:], pr[:])
    nc.compile()
    return nc


def _extract_weights(inputs):
    f = lambda a: [float(v) for v in np.asarray(a).reshape(-1)]
    return dict(
        w00=f(inputs["c0_w"])[0], w01=f(inputs["c0_w"])[1], b0=f(inputs["c0_b"])[0],
        w2=f(inputs["c2_w"]), b2=f(inputs["c2_b"])[0],
        w4=f(inputs["c4_w"]), b4=f(inputs["c4_b"])[0],
        w6=f(inputs["c6_w"]), b6=f(inputs["c6_b"])[0],
        w8=f(inputs["c8_w"]), b8=f(inputs["c8_b"])[0],
        cls_b=f(inputs["cls_b"]),
    )


def _make_clsT(cls_w):
    """Classifier weights in the device layout. Consecutive-row mapping:
    feat[p, s, c] is dram row 64p + 8s + c -> node 64*(p%16) + 8s + c,
    batch block b = p//16; cols 192:200 hold the 0/1 block mask."""
    clsT = np.zeros((128, 200), np.float32)
    pidx = np.arange(128)
    node = (64 * (pidx % 16))[:, None] + np.arange(64)[None, :]   # [p, s*8+c]
    for j in range(3):
        clsT[:, j * 64:(j + 1) * 64] = cls_w[j][node]
    clsT[pidx, 192 + pidx // 16] = 1.0
    return clsT


def _prep_x(x):
    """fp32 (BS*NN, T) -> fp16 with each row permuted into 6 phase planes:
    col = (2j+e)*200 + g for original t = 6g + 2j + e."""
    x16 = np.asarray(x, dtype=np.float32).reshape(BS * NN, 200, 3, 2)
    x16 = x16.transpose(0, 2, 3, 1).astype(np.float16)
    return np.ascontiguousarray(x16).reshape(BS * NN, T)


def _run(inputs, trace=False, trace_kwargs=None, allow_const=True):
    w = _extract_weights(inputs)
    const_feat = _interval_const_feat(w) if allow_const else None
    w["const_feat"] = const_feat
    cls_w_ = np.asarray(inputs["cls_w"], dtype=np.float32)
    if const_feat is not None:
        # weight-only logit bound: safe to skip softmax's max-subtract?
        lg_host = const_feat * cls_w_.astype(np.float64).sum(axis=1) \
            + np.asarray(w["cls_b"], np.float64)
        w["lg_bounded"] = bool(np.all(np.isfinite(lg_host))
                               and np.abs(lg_host).max() < 60.0)
        w["wst_narrow"] = bool(abs(const_feat) > 1e-20
                               and np.isfinite(3.0 / const_feat))
    key = tuple(np.asarray(
        [w["w00"], w["w01"], w["b0"]] + w["w2"] + [w["b2"]] + w["w4"] + [w["b4"]]
        + w["w6"] + [w["b6"]] + w["w8"] + [w["b8"]] + w["cls_b"]
        + [0.0 if const_feat is None else
           (2.0 if w.get("lg_bounded") else 1.0)
           + (4.0 if w.get("wst_narrow") else 0.0)],
        np.float64
    ).tobytes())
    if key not in _CACHE:
        _CACHE[key] = _build_const(w) if const_feat is not None else _build(w)
    nc = _CACHE[key]

    if const_feat is not None:
        # conv pyramid proven constant for ALL inputs with these weights:
        # device computes classifier + softmax only.
        if w["wst_narrow"]:
            wsm = np.zeros((16, 195), np.float32)
            wj = cls_w_.reshape(3, 16, 64).transpose(1, 0, 2)   # [p, j, c]
            for j in range(3):
                wsm[:, 65 * j:65 * j + 64] = wj[:, j, :]
                wsm[:, 65 * j + 64] = np.float32(
                    w["cls_b"][j] / (16.0 * const_feat))
        else:
            wsm = np.ascontiguousarray(
                cls_w_.reshape(3, 128, 8).transpose(1, 2, 0).reshape(128, 24))
        in_maps = [{"wsmt": wsm} for _ in range(N_CORES)]
        res = run_bass_kernel_spmd(nc, in_maps, list(range(N_CORES)),
                                   trace=trace, **(trace_kwargs or {}))
        out = np.concatenate(
            [np.asarray(res.results[i]["out"]) for i in range(N_CORES)],
            axis=0).astype(np.float32)
        return out, res

    xp = _prep_x(np.asarray(inputs["x"], dtype=np.float32).reshape(BS * NN, T))

    # scaled-identity stationaries: w0*I, w1*I, w2_k*I
    wid = np.zeros((128, 6 * 128), np.float16)
    ar = np.arange(128)
    for k, val in enumerate([w["w00"], w["w01"]] + list(w["w2"])):
        wid[ar, 128 * k + ar] = np.float16(val)

    clsT = _make_clsT(cls_w_)

    rows_per_core = BS * NN // N_CORES
    in_maps = [
        {"x": np.ascontiguousarray(xp[i * rows_per_core:(i + 1) * rows_per_core]),
         "wid": wid, "clswt": clsT}
        for i in range(N_CORES)
    ]
    res = run_bass_kernel_spmd(nc, in_maps, list(range(N_CORES)), trace=trace,
                               **(trace_kwargs or {}))
    out = np.concatenate([np.asarray(res.results[i]["out"]) for i in range(N_CORES)],
                         axis=0).astype(np.float32)
    return out, res


def kernel(**inputs):
    out, _ = _run(inputs, trace=False)
    return out
